# revision 33
# baseline (speedup 1.0000x reference)
"""Trainium2 Bass kernel for MultiHeadDifferentialAttention.

Strategy: data-parallel over batch. B=8 batches map 1:1 onto the 8
NeuronCores; each core runs the full per-batch pipeline (QKV proj ->
differential attention -> LayerNorm -> output proj) with no collectives.
The host pre-lays-out inputs (x transposed per batch, weights reshaped
into partition-major tiles, gamma/beta/0.8 folded into Wp/bp) and
transposes the per-core [768, 1024] outputs back at the end.

Device pipeline per core:
  - ~10 dummy warm-up matmuls on a zeroed tile run while the input DMAs
    land, so the PE HAM clock-gate reaches K=8/8 (2.4 GHz) before the
    real stream starts instead of ~8us into it.
  - v = x @ Wv (fp16 operands, fp32 accum) into an augmented layout
    [tok, head, 128+1] whose last column is ones, so the attention-value
    matmul also produces the softmax denominator (column 128) for free.
  - qT/kT = (x @ Wq)^T per head in [2D=128, tok] fp16 layout: q1/q2 land
    on partitions 0-63 / 64-127, so the two K=64 score matmuls pack into
    disjoint PE row groups and run concurrently (they must target
    different PSUM banks - concurrent same-bank PE writes fault).
  - scores S^T[m, n] on PSUM -> one strided exp per m on ScalarE (scale
    fused) -> fp16 E tiles.
  - AV: E tile is the stationary operand, rhs = [v_h | 1]; out[n, 0:128]
    is the unnormalized attention output, out[:, 128] the denominator.
    The two scores' accumulation chains share one PSUM bank
    (only the first matmul carries start=True - start clears the
    has_written bits bank-wide) and run un-interleaved so LDW/MM pairs
    pipeline.
  - combine a1 - lam*a2 and LayerNorm on VectorE, all per-partition.
    rsqrt = exp(-0.5*ln(var+eps)) on ScalarE: the activation-table patch
    below pins exp and ln to the one table set containing both, so the
    per-head LayerNorm causes no table reloads. The finished head is
    immediately transposed into the [1536, tok] layout by the DMA XBAR
    (SBUF->SBUF dma_start(transpose=True)), NOT the PE - this keeps the
    TensorE free and avoids the HAM re-throttle the transpose phase
    used to cause. Output is F^T [768, 1024].
  - final projection: per output tile [128 x 512], contract over all 12
    heads k-inner (k order rotated per tile so the last head's arrival
    stalls nothing), evict+DMA per tile so the tail pipeline drains
    while later tiles still stream on the PE.
"""

import numpy as np

B, N, C, H = 8, 1024, 768, 12
D = C // H  # 64
TD = 2 * D  # 128
LAMBDA_INIT = 0.8 - 0.6 * np.exp(-0.3 * (1 - 1))  # 0.2
OUT_SCALE = 1.0 - LAMBDA_INIT  # 0.8
EPS = 1e-5
SCALE = float(D) ** -0.5  # 1/8

_BUILD_CACHE = {}
LAST_EXEC_NS = None


def _patch_act_tables(mybir, bacc):
    """Pin Exp and Ln to natural_log_exp_and_others so interleaving them
    never reloads the ScalarE spline tables."""
    from concourse import hw_specs

    orig = hw_specs.get_activation_tables
    if getattr(bacc.get_activation_tables, "_nlx_pinned", False):
        return

    def patched(arch):
        tables = orig(arch)
        exp = mybir.ActivationFunctionType.Exp
        ln = mybir.ActivationFunctionType.Ln
        for name, funcs in tables.items():
            if name != "natural_log_exp_and_others":
                funcs.discard(exp)
                funcs.discard(ln)
        return tables

    patched._nlx_pinned = True
    bacc.get_activation_tables = patched


def _build(lam: float, dbg: bool = False):
    import concourse.bass as bass  # noqa: F401
    import concourse.mybir as mybir
    import concourse.tile as tile
    from concourse import bacc

    _patch_act_tables(mybir, bacc)

    f32 = mybir.dt.float32
    f16 = mybir.dt.float16
    AF = mybir.ActivationFunctionType
    OP = mybir.AluOpType

    nc = bacc.Bacc(None, target_bir_lowering=False, debug=False)

    XT = nc.declare_dram_parameter("xT", [128, 6, 1024], f16, isOutput=False)
    WQR = nc.declare_dram_parameter("WqR", [12, 128, 6, 128], f16, isOutput=False)
    WKR = nc.declare_dram_parameter("WkR", [12, 128, 6, 128], f16, isOutput=False)
    WVR = nc.declare_dram_parameter("WvR", [128, 6, 1536], f16, isOutput=False)
    WPR = nc.declare_dram_parameter("WpR", [12, 128, 768], f16, isOutput=False)
    BPP = nc.declare_dram_parameter("bpp", [128, 6], f32, isOutput=False)
    OUT = nc.declare_dram_parameter("outT", [128, 6, 1024], f32, isOutput=True)
    if dbg:
        DVAUG = nc.declare_dram_parameter("d_vaug", [128, 8, 12, 129], f16, isOutput=True)
        DQH = nc.declare_dram_parameter("d_qh", [128, 1024], f16, isOutput=True)
        DKH = nc.declare_dram_parameter("d_kh", [128, 1024], f16, isOutput=True)
        DE12 = nc.declare_dram_parameter("d_e12", [128, 8, 1024], f16, isOutput=True)
        DOLN = nc.declare_dram_parameter("d_oln", [128, 12, 8, 128], f16, isOutput=True)
        DSTATS = nc.declare_dram_parameter("d_stats", [128, 12, 8, 2], f32, isOutput=True)
        DOLNT = nc.declare_dram_parameter("d_olnT", [128, 12, 1024], f16, isOutput=True)

    with tile.TileContext(nc) as tc:
        with tc.tile_pool(name="persist", bufs=1) as persist:
            o_ln = persist.tile([128, 12, 8, 128], f16)
            o_lnT = persist.tile([128, 12, 1024], f16)
            stats_sb = persist.tile([128, 12, 8, 2], f32)
            sdbuf = persist.tile([128, 12, 8], f32)
            rsbuf = persist.tile([128, 12, 8], f32)
            bpp_sb = persist.tile([128, 6], f32)
            eps_sb = persist.tile([128, 1], f32)
            warm = persist.tile([128, 512], f16)
            wpks = [
                persist.tile([128, 768], f16, name=f"wpk{k}") for k in range(12)
            ]

            # PE warm-up: the HAM clock gate keeps the PE at 1.2 GHz until
            # it has been busy one full 3.4us activity window.  Burn that
            # window on dummy matmuls while the first input DMAs are in
            # flight so the real stream starts at 2.4 GHz.  GpSimd is the
            # first engine out of the preamble barriers, so it memsets.
            nc.gpsimd.memset(warm[:], 0.0)
            with tc.tile_pool(name="warmps", bufs=1, space="PSUM") as warmp:
                wps = warmp.tile([128, 512], f32)
                for _ in range(10):
                    nc.tensor.matmul(
                        wps[:], warm[:, 0:128], warm[:], start=True, stop=True
                    )

            with tc.tile_pool(name="longA", bufs=1) as longA:
                xTk = [
                    longA.tile([128, 1024], f16, name=f"xT{k}") for k in range(6)
                ]
                v_aug = longA.tile([128, 8, 12, 129], f16)

                from contextlib import ExitStack as _ES
                _pools = _ES()
                wqkp = _pools.enter_context(tc.tile_pool(name="wqk", bufs=6))
                qkp = _pools.enter_context(tc.tile_pool(name="qk", bufs=3))
                qkps = _pools.enter_context(
                    tc.tile_pool(name="qkps", bufs=2, space="PSUM")
                )

                qk_w = {}

                def issue_qk_dma(h):
                    """Issue w_q/w_k DMAs for head h (prefetched several
                    heads ahead so the projection LDWs never wait on DMA)."""
                    wqh = wqkp.tile([128, 6, 128], f16, tag="wq",
                                    name=f"wqh{h}")
                    wkh = wqkp.tile([128, 6, 128], f16, tag="wk",
                                    name=f"wkh{h}")
                    nc.sync.dma_start(out=wqh[:], in_=WQR[h])
                    nc.sync.dma_start(out=wkh[:], in_=WKR[h])
                    qk_w[h] = (wqh, wkh)

                def emit_qk(h):
                    """Project q^T/k^T for head h from prefetched weights."""
                    wqh, wkh = qk_w.pop(h)
                    qh = qkp.tile([128, 1024], f16, tag="q", name=f"qh{h}")
                    kh = qkp.tile([128, 1024], f16, tag="k", name=f"kh{h}")
                    for which, (wt, dst) in enumerate(((wqh, qh), (wkh, kh))):
                        ps0 = qkps.tile([128, 512], f32, tag="qk",
                                        name=f"ps0_{which}")
                        ps1 = qkps.tile([128, 512], f32, tag="qk",
                                        name=f"ps1_{which}")
                        for k in range(6):
                            nc.tensor.matmul(
                                ps0[:], wt[:, k, :], xTk[k][:, 0:512],
                                start=(k == 0), stop=(k == 5),
                            )
                            nc.tensor.matmul(
                                ps1[:], wt[:, k, :], xTk[k][:, 512:1024],
                                start=(k == 0), stop=(k == 5),
                            )
                        nc.vector.tensor_copy(dst[:, 0:512], ps0[:])
                        nc.vector.tensor_copy(dst[:, 512:1024], ps1[:])
                    return qh, kh

                # ---- Phase 1: v = x @ Wv into v_aug ----
                with (
                    tc.tile_pool(name="wv", bufs=1) as wvp,
                    tc.tile_pool(name="vps", bufs=2, space="PSUM") as vps,
                ):
                    wvk = [
                        wvp.tile([128, 1536], f16, name=f"wv{k}")
                        for k in range(6)
                    ]
                    # critical-path DMAs first: the first v matmul needs
                    # xT0 (stationary) + wv0 (moving); head 0's q/k weights
                    # come right after so the qk projection that follows the
                    # v projection is never starved.  bpp goes on the scalar
                    # HWDGE queue - it is only read at the very end.
                    for k in range(6):
                        nc.sync.dma_start(out=xTk[k][:], in_=XT[:, k])
                        nc.sync.dma_start(out=wvk[k][:], in_=WVR[:, k])
                    for hh in range(4):
                        issue_qk_dma(hh)
                    nc.vector.memset(v_aug[:, :, :, 128:129], 1.0)
                    nc.vector.memset(eps_sb[:], EPS)
                    nc.scalar.dma_start(out=bpp_sb[:], in_=BPP[:])
                    next_qk = emit_qk(0)
                    # prefetch the projection weights NOW: these plain DMAs
                    # must not be in flight alongside the XBAR transposes
                    # later (observed data corruption when mixed).
                    for k in range(12):
                        nc.sync.dma_start(out=wpks[k][:], in_=WPR[k])
                    for t in range(8):
                        # one stationary xT chunk serves all three c-ranges
                        pss = [
                            vps.tile([128, 512], f32, tag=f"vps{cr}",
                                     name=f"vps{cr}")
                            for cr in range(3)
                        ]
                        for k in range(6):
                            for cr in range(3):
                                nc.tensor.matmul(
                                    pss[cr][:],
                                    xTk[k][:, t * 128 : (t + 1) * 128],
                                    wvk[k][:, cr * 512 : (cr + 1) * 512],
                                    start=(k == 0),
                                    stop=(k == 5),
                                )
                        for cr in range(3):
                            nc.scalar.copy(
                                v_aug[:, t, 4 * cr : 4 * cr + 4, 0:128],
                                pss[cr][:].rearrange("p (h c) -> p h c", c=128),
                            )

                # ---- Phase 2: attention per head, tail fused per head ----
                with (
                    tc.tile_pool(name="estrip", bufs=3) as ep,
                    tc.tile_pool(name="fin", bufs=4) as fin,
                    tc.tile_pool(name="spool", bufs=2, space="PSUM") as spool,
                    tc.tile_pool(name="avps", bufs=2, space="PSUM") as avps,
                ):

                    av_state = {}

                    def av_chain(ph, pr, e12p, idx):
                        """Emit ONE of the 8 AV accumulation chains of strip
                        (ph, pr).  Chains are emitted interleaved between
                        score pairs so the score LDWs always have AV matmuls
                        behind them to hide their weight-buffer turnaround.
                        idx: 2*c2 + (0: E1 chain, 1: E2 chain + vector tail).
                        """
                        c2, second = divmod(idx, 2)
                        jn = pr * 4 + c2
                        if not second:
                            o = avps.tile([128, 258], f32, tag="o",
                                          name="o_av")
                            av_state[c2] = o
                            # Both accumulation chains share one PSUM bank.
                            # start=True clears has_written bank-wide, so
                            # only the very first matmul may set it.
                            for m in range(8):
                                nc.tensor.matmul(
                                    o[:, 0:129],
                                    e12p[:, m, c2 * 128 : (c2 + 1) * 128],
                                    v_aug[:, m, ph, :],
                                    start=(m == 0),
                                    stop=(m == 7),
                                    skip_group_check=True,
                                )
                            return
                        o = av_state[c2]
                        for m in range(8):
                            nc.tensor.matmul(
                                o[:, 129:258],
                                e12p[:, m, 512 + c2 * 128 : 512 + (c2 + 1) * 128],
                                v_aug[:, m, ph, :],
                                start=False,
                                stop=(m == 7),
                                skip_group_check=True,
                            )
                        # combine + LN stats (VectorE, all per-partition).
                        # Each DVE op may read PSUM at most once, which
                        # forces the two-step r1*u1 - (lam*r2)*u2 form.
                        r1 = fin.tile([128, 1], f32, tag="r1")
                        r2 = fin.tile([128, 1], f32, tag="r2")
                        nc.vector.reciprocal(r1[:], o[:, 128:129])
                        nc.vector.reciprocal(r2[:], o[:, 257:258])
                        t2 = fin.tile([128, 128], f32, tag="t2")
                        nc.vector.tensor_scalar(
                            t2[:], o[:, 129:257], r2[:], float(lam),
                            op0=OP.mult, op1=OP.mult,
                        )
                        nc.vector.scalar_tensor_tensor(
                            o_ln[:, ph, jn, :],
                            o[:, 0:128],
                            r1[:],
                            t2[:],
                            op0=OP.mult,
                            op1=OP.subtract,
                        )
                        st6 = fin.tile([128, 6], f32, tag="st6")
                        nc.vector.bn_stats(st6[:], o_ln[:, ph, jn, :])
                        nc.vector.bn_aggr(stats_sb[:, ph, jn, :], st6[:])

                    def head_tail(ph):
                        """rsqrt, LN apply, transpose for a finished head.
                        rs = exp(-0.5 * ln(var + eps)); Exp and Ln share one
                        pinned table set, so no reload happens here."""
                        nc.scalar.activation(
                            sdbuf[:, ph, :], stats_sb[:, ph, :, 1],
                            AF.Ln, bias=eps_sb[:],
                        )
                        nc.scalar.activation(
                            rsbuf[:, ph, :], sdbuf[:, ph, :], AF.Exp,
                            scale=-0.5,
                        )
                        for jn in range(8):
                            nc.vector.tensor_scalar(
                                o_ln[:, ph, jn, :],
                                o_ln[:, ph, jn, :],
                                stats_sb[:, ph, jn, 0:1],
                                rsbuf[:, ph, jn : jn + 1],
                                op0=OP.subtract,
                                op1=OP.mult,
                            )
                        # transpose the whole finished head [tok, (jn, c)] ->
                        # [c, (jn, tok)] in ONE XBAR DMA (blocked transpose:
                        # out[c, jn, t] = in[t, jn, c]) - TensorE never sees
                        # it, and one instruction per head keeps the sync
                        # queue issue cost negligible.
                        nc.sync.dma_start(
                            out=o_lnT[:, ph, :].rearrange(
                                "p (a b) -> p a b", b=128
                            ),
                            in_=o_ln[:, ph],
                            transpose=True,
                        )

                    pending = None  # (head, strip, e12) awaiting its AV
                    for h in range(12):
                        qh, kh = next_qk

                        for r in range(2):
                            e12 = ep.tile([128, 8, 1024], f16, tag="e")
                            nsl = slice(r * 512, (r + 1) * 512)
                            for m in range(8):
                                msl = slice(m * 128, (m + 1) * 128)
                                # The two score matmuls must hit different
                                # PSUM banks (concurrent row-group writes to
                                # one bank fault); one exp covers both.
                                sp = spool.tile([128, 2, 512], f32, tag="s")
                                nc.tensor.matmul(
                                    sp[:, 0, :], kh[0:64, msl], qh[0:64, nsl],
                                    start=True, stop=True,
                                )
                                nc.tensor.matmul(
                                    sp[:, 1, :], kh[64:128, msl],
                                    qh[64:128, nsl],
                                    start=True, stop=True,
                                )
                                if pending is not None:
                                    av_chain(pending[0], pending[1],
                                             pending[2], m)
                                nc.scalar.activation(
                                    e12[:, m, :].rearrange("p (a b) -> p a b", a=2),
                                    sp[:],
                                    AF.Exp,
                                    scale=SCALE,
                                )
                            if dbg and h == 0 and r == 0:
                                nc.sync.dma_start(out=DE12[:], in_=e12[:])
                            # emit_qk BEFORE head_tail: the transpose DMA in
                            # head_tail blocks the in-order sync queue until
                            # its head's LN applies land, which would delay
                            # the next head's weight DMAs behind it.
                            if r == 0 and h + 1 < 12:
                                if h + 4 < 12:
                                    issue_qk_dma(h + 4)
                                # next head's q/k projection fills PE bubbles
                                # while ScalarE chews this strip's exp
                                next_qk = emit_qk(h + 1)
                            if pending is not None and pending[1] == 1:
                                head_tail(pending[0])
                            pending = (h, r, e12)
                        if dbg and h == 0:
                            nc.sync.dma_start(out=DQH[:], in_=qh[:])
                            nc.sync.dma_start(out=DKH[:], in_=kh[:])

                    # drain: AV + tail of the very last strip
                    for m in range(8):
                        av_chain(pending[0], pending[1], pending[2], m)
                    head_tail(pending[0])
                    if dbg:
                        nc.sync.dma_start(out=DVAUG[:], in_=v_aug[:])
                        nc.sync.dma_start(out=DOLN[:], in_=o_ln[:])
                        nc.sync.dma_start(out=DSTATS[:], in_=stats_sb[:])

                _pools.close()

            # longA (xT, v_aug) released here.
            if dbg:
                nc.sync.dma_start(out=DOLNT[:], in_=o_lnT[:])
            # ---- Phase 3: final projection, one [128, 512] tile at a time.
            # Wave A (first 6 tiles) streams heads 0..10 and DEFERS head 11,
            # so the PE has ~14us of proj work queued before it ever needs
            # the last head's transpose; wave B then runs all heads k-inner
            # per tile so evictions/output DMAs pipeline behind the stream.
            with tc.tile_pool(name="tail", bufs=1) as tailp:
                fout = tailp.tile([128, 6, 1024], f32)
                with (
                    tc.tile_pool(name="fpsA", bufs=1, space="PSUM") as fpsA,
                    tc.tile_pool(name="fpsB", bufs=2, space="PSUM") as fpsB,
                ):
                    tiles = [(mc, nr2) for mc in range(6) for nr2 in range(2)]

                    def evict(fs, mc, nr2, scalar_side=False):
                        # Alternate evict+DMA between the DVE/sync and
                        # ScalarE/scalar queue pairs so the final drain
                        # pipelines across two queues.  AF.Copy + bias AP
                        # makes ScalarE do the bias add.
                        nsl2 = slice(nr2 * 512, (nr2 + 1) * 512)
                        if scalar_side:
                            nc.scalar.activation(
                                fout[:, mc, nsl2], fs[:], AF.Identity,
                                bias=bpp_sb[:, mc : mc + 1],
                            )
                            nc.scalar.dma_start(
                                out=OUT[:, mc, nsl2],
                                in_=fout[:, mc, nsl2],
                            )
                        else:
                            nc.vector.tensor_scalar(
                                fout[:, mc, nsl2],
                                fs[:],
                                bpp_sb[:, mc : mc + 1],
                                None,
                                op0=OP.add,
                            )
                            nc.sync.dma_start(
                                out=OUT[:, mc, nsl2],
                                in_=fout[:, mc, nsl2],
                            )

                    fsA = [
                        fpsA.tile([128, 512], f32, tag=f"fa{t}", name=f"fA{t}")
                        for t in range(6)
                    ]
                    for t in range(6):
                        mc, nr2 = tiles[t]
                        nsl2 = slice(nr2 * 512, (nr2 + 1) * 512)
                        ks = [(k0 + t) % 11 for k0 in range(11)]
                        for i, k in enumerate(ks):
                            nc.tensor.matmul(
                                fsA[t][:],
                                wpks[k][:, mc * 128 : (mc + 1) * 128],
                                o_lnT[:, k, nsl2],
                                start=(i == 0),
                                stop=False,
                            )
                    for t in range(6):
                        mc, nr2 = tiles[t]
                        nsl2 = slice(nr2 * 512, (nr2 + 1) * 512)
                        nc.tensor.matmul(
                            fsA[t][:],
                            wpks[11][:, mc * 128 : (mc + 1) * 128],
                            o_lnT[:, 11, nsl2],
                            start=False,
                            stop=True,
                        )
                        evict(fsA[t], mc, nr2)
                    for t in range(6, 12):
                        mc, nr2 = tiles[t]
                        nsl2 = slice(nr2 * 512, (nr2 + 1) * 512)
                        fs = fpsB.tile([128, 512], f32, tag="fb",
                                       name=f"fB{t}")
                        ks = [(k0 + t) % 12 for k0 in range(12)]
                        for i, k in enumerate(ks):
                            nc.tensor.matmul(
                                fs[:],
                                wpks[k][:, mc * 128 : (mc + 1) * 128],
                                o_lnT[:, k, nsl2],
                                start=(i == 0),
                                stop=(i == 11),
                            )
                        evict(fs, mc, nr2, scalar_side=(t % 2 == 1))

    nc.compile()
    return nc


def _host_prep(x, Wq, Wk, Wv, gamma, beta, Wp, bp):
    x = np.ascontiguousarray(np.asarray(x, np.float32))
    Wq = np.asarray(Wq, np.float32)
    Wk = np.asarray(Wk, np.float32)
    Wv = np.asarray(Wv, np.float32)
    Wp = np.asarray(Wp, np.float32)
    bp = np.asarray(bp, np.float32)
    gamma = np.asarray(gamma, np.float32)
    beta = np.asarray(beta, np.float32)

    # xT per batch: [128, 6, 1024] with [p, k, n] = x[b, n, k*128+p]
    xTr = np.ascontiguousarray(
        x.transpose(0, 2, 1).reshape(B, 6, 128, N).transpose(0, 2, 1, 3)
    ).astype(np.float16)

    # W[qk]R: [12, 128, 6, 128] with [h, p, k, c] = W[k*128+p, h*128+c]
    def wqk_r(W):
        return np.ascontiguousarray(
            W.reshape(6, 128, 12, 128).transpose(2, 1, 0, 3)
        )

    WqR = wqk_r(Wq).astype(np.float16)
    WkR = wqk_r(Wk).astype(np.float16)
    # WvR: [128, 6, 1536] with [p, k, c] = Wv[k*128+p, c]
    WvR = np.ascontiguousarray(
        Wv.reshape(6, 128, 2 * C).transpose(1, 0, 2)
    ).astype(np.float16)
    # Fold gamma and the (1 - lambda_init) scale into Wp; beta into the bias.
    gfull = np.tile(gamma, H)  # [1536]
    Wpg = Wp * (OUT_SCALE * gfull)[:, None]
    bpp = bp + OUT_SCALE * (np.tile(beta, H) @ Wp)
    WpR = np.ascontiguousarray(Wpg.reshape(12, 128, C)).astype(np.float16)
    bppR = np.ascontiguousarray(bpp.reshape(6, 128).T)  # [128, 6]
    return xTr, WqR, WkR, WvR, WpR, bppR


def kernel(x, Wq, Wk, Wv, lam, gamma, beta, Wp, bp):
    global LAST_EXEC_NS
    import os

    from concourse.bass_utils import run_bass_kernel_spmd

    lam_f = float(np.asarray(lam))
    xTr, WqR, WkR, WvR, WpR, bppR = _host_prep(
        x, Wq, Wk, Wv, gamma, beta, Wp, bp
    )

    key = lam_f
    if key not in _BUILD_CACHE:
        _BUILD_CACHE[key] = _build(lam_f)
    nc = _BUILD_CACHE[key]

    in_maps = [
        {
            "xT": xTr[b],
            "WqR": WqR,
            "WkR": WkR,
            "WvR": WvR,
            "WpR": WpR,
            "bpp": bppR,
        }
        for b in range(B)
    ]

    trace = bool(os.environ.get("BASS_KERNEL_TRACE"))
    if trace:
        from concourse import bass_utils as _bu

        _bu.upload_artifacts = lambda tmpdir: "local://" + tmpdir
    res = run_bass_kernel_spmd(
        nc, in_maps, list(range(B)), trace=trace,
        **({"trace_cores": list(range(B))} if trace else {}),
    )
    LAST_EXEC_NS = res.exec_time_ns

    out = np.empty((B, N, C), np.float32)
    for b in range(B):
        outT = res.results[b]["outT"]  # [128, 6, 1024]
        out[b] = outT.transpose(2, 1, 0).reshape(N, C)
    return out


# revision 36
# speedup vs baseline: 1.1958x; 1.1958x over previous
"""Trainium2 Bass kernel for MultiHeadDifferentialAttention.

Strategy: data-parallel over batch. B=8 batches map 1:1 onto the 8
NeuronCores; each core runs the full per-batch pipeline (QKV proj ->
differential attention -> LayerNorm -> output proj) with no collectives.
The host pre-lays-out inputs (x transposed per batch, weights reshaped
into partition-major tiles, gamma/beta/0.8 folded into Wp/bp) and
transposes the per-core [768, 1024] outputs back at the end.

Device pipeline per core:
  - ~10 dummy warm-up matmuls on a zeroed tile run while the input DMAs
    land, so the PE HAM clock-gate reaches K=8/8 (2.4 GHz) before the
    real stream starts instead of ~8us into it.
  - v = x @ Wv (fp16 operands, fp32 accum) into an augmented layout
    [tok, head, 128+1] whose last column is ones, so the attention-value
    matmul also produces the softmax denominator (column 128) for free.
  - qT/kT = (x @ Wq)^T per head in [2D=128, tok] fp16 layout: q1/q2 land
    on partitions 0-63 / 64-127, so the two K=64 score matmuls pack into
    disjoint PE row groups and run concurrently (they must target
    different PSUM banks - concurrent same-bank PE writes fault).
  - scores S^T[m, n] on PSUM -> one strided exp per m on ScalarE (scale
    fused) -> fp16 E tiles.
  - AV: E tile is the stationary operand, rhs = [v_h | 1]; out[n, 0:128]
    is the unnormalized attention output, out[:, 128] the denominator.
    The two scores' accumulation chains share one PSUM bank
    (only the first matmul carries start=True - start clears the
    has_written bits bank-wide) and run un-interleaved so LDW/MM pairs
    pipeline.
  - combine a1 - lam*a2 and LayerNorm on VectorE, all per-partition.
    rsqrt = exp(-0.5*ln(var+eps)) on ScalarE: the activation-table patch
    below pins exp and ln to the one table set containing both, so the
    per-head LayerNorm causes no table reloads. The finished head is
    immediately transposed into the [1536, tok] layout by the DMA XBAR
    (SBUF->SBUF dma_start(transpose=True)), NOT the PE - this keeps the
    TensorE free and avoids the HAM re-throttle the transpose phase
    used to cause. Output is F^T [768, 1024].
  - final projection: per output tile [128 x 512], contract over all 12
    heads k-inner (k order rotated per tile so the last head's arrival
    stalls nothing), evict+DMA per tile so the tail pipeline drains
    while later tiles still stream on the PE.
"""

import numpy as np

B, N, C, H = 8, 1024, 768, 12
D = C // H  # 64
TD = 2 * D  # 128
LAMBDA_INIT = 0.8 - 0.6 * np.exp(-0.3 * (1 - 1))  # 0.2
OUT_SCALE = 1.0 - LAMBDA_INIT  # 0.8
EPS = 1e-5
SCALE = float(D) ** -0.5  # 1/8

_BUILD_CACHE = {}
LAST_EXEC_NS = None


def _patch_act_tables(mybir, bacc):
    """Pin Exp and Ln to natural_log_exp_and_others so interleaving them
    never reloads the ScalarE spline tables."""
    from concourse import hw_specs

    orig = hw_specs.get_activation_tables
    if getattr(bacc.get_activation_tables, "_nlx_pinned", False):
        return

    def patched(arch):
        tables = orig(arch)
        exp = mybir.ActivationFunctionType.Exp
        ln = mybir.ActivationFunctionType.Ln
        for name, funcs in tables.items():
            if name != "natural_log_exp_and_others":
                funcs.discard(exp)
                funcs.discard(ln)
        return tables

    patched._nlx_pinned = True
    bacc.get_activation_tables = patched


def _build(lam: float, dbg: bool = False):
    import concourse.bass as bass  # noqa: F401
    import concourse.mybir as mybir
    import concourse.tile as tile
    from concourse import bacc

    _patch_act_tables(mybir, bacc)

    f32 = mybir.dt.float32
    f16 = mybir.dt.float16
    AF = mybir.ActivationFunctionType
    OP = mybir.AluOpType

    nc = bacc.Bacc(None, target_bir_lowering=False, debug=False)

    XT = nc.declare_dram_parameter("xT", [128, 6, 1024], f16, isOutput=False)
    WQR = nc.declare_dram_parameter("WqR", [12, 128, 6, 128], f16, isOutput=False)
    WKR = nc.declare_dram_parameter("WkR", [12, 128, 6, 128], f16, isOutput=False)
    WVR = nc.declare_dram_parameter("WvR", [128, 6, 1536], f16, isOutput=False)
    WPR = nc.declare_dram_parameter("WpR", [12, 128, 768], f16, isOutput=False)
    BPP = nc.declare_dram_parameter("bpp", [128, 6], f32, isOutput=False)
    OUT = nc.declare_dram_parameter("outT", [128, 6, 1024], f32, isOutput=True)
    if dbg:
        DVAUG = nc.declare_dram_parameter("d_vaug", [128, 8, 12, 129], f16, isOutput=True)
        DQH = nc.declare_dram_parameter("d_qh", [128, 1024], f16, isOutput=True)
        DKH = nc.declare_dram_parameter("d_kh", [128, 1024], f16, isOutput=True)
        DE12 = nc.declare_dram_parameter("d_e12", [128, 8, 1024], f16, isOutput=True)
        DOLN = nc.declare_dram_parameter("d_oln", [128, 12, 8, 128], f16, isOutput=True)
        DSTATS = nc.declare_dram_parameter("d_stats", [128, 12, 8, 2], f32, isOutput=True)
        DOLNT = nc.declare_dram_parameter("d_olnT", [128, 12, 1024], f16, isOutput=True)

    with tile.TileContext(nc) as tc:
        with tc.tile_pool(name="persist", bufs=1) as persist:
            o_ln = persist.tile([128, 12, 8, 128], f16)
            o_lnT = persist.tile([128, 12, 1024], f16)
            stats_sb = persist.tile([128, 12, 8, 2], f32)
            sdbuf = persist.tile([128, 12, 8], f32)
            rsbuf = persist.tile([128, 12, 8], f32)
            bpp_sb = persist.tile([128, 6], f32)
            eps_sb = persist.tile([128, 1], f32)
            warm = persist.tile([128, 512], f16)
            wpks = [
                persist.tile([128, 768], f16, name=f"wpk{k}") for k in range(12)
            ]

            # PE warm-up: the HAM clock gate keeps the PE at 1.2 GHz until
            # it has been busy one full 3.4us activity window.  Burn that
            # window on dummy matmuls while the first input DMAs are in
            # flight so the real stream starts at 2.4 GHz.  GpSimd is the
            # first engine out of the preamble barriers, so it memsets.
            nc.gpsimd.memset(warm[:], 0.0)
            with tc.tile_pool(name="warmps", bufs=1, space="PSUM") as warmp:
                wps = warmp.tile([128, 512], f32)
                for _ in range(10):
                    nc.tensor.matmul(
                        wps[:], warm[:, 0:128], warm[:], start=True, stop=True
                    )

            with tc.tile_pool(name="longA", bufs=1) as longA:
                xTk = [
                    longA.tile([128, 1024], f16, name=f"xT{k}") for k in range(6)
                ]
                v_aug = longA.tile([128, 8, 12, 129], f16)

                from contextlib import ExitStack as _ES
                _pools = _ES()
                wqkp = _pools.enter_context(tc.tile_pool(name="wqk", bufs=6))
                qkp = _pools.enter_context(tc.tile_pool(name="qk", bufs=3))
                qkps = _pools.enter_context(
                    tc.tile_pool(name="qkps", bufs=2, space="PSUM")
                )

                qk_w = {}

                def issue_qk_dma(h):
                    """Issue w_q/w_k DMAs for head h (prefetched several
                    heads ahead so the projection LDWs never wait on DMA)."""
                    wqh = wqkp.tile([128, 6, 128], f16, tag="wq",
                                    name=f"wqh{h}")
                    wkh = wqkp.tile([128, 6, 128], f16, tag="wk",
                                    name=f"wkh{h}")
                    nc.sync.dma_start(out=wqh[:], in_=WQR[h])
                    nc.sync.dma_start(out=wkh[:], in_=WKR[h])
                    qk_w[h] = (wqh, wkh)

                def emit_qk(h):
                    """Return (qh, kh) tiles plus a generator that emits the
                    projection matmuls one at a time.  The caller pulls steps
                    between score pairs / AV chains so the qk matmuls spread
                    deterministically across the head's 16 m-slots - that
                    spacing is what gives ScalarE's exp stream time to keep
                    ahead of the score PSUM reuse (spool bufs=2)."""
                    wqh, wkh = qk_w.pop(h)
                    qh = qkp.tile([128, 1024], f16, tag="q", name=f"qh{h}")
                    kh = qkp.tile([128, 1024], f16, tag="k", name=f"kh{h}")

                    def gen():
                        for which, (wt, dst) in enumerate(
                            ((wqh, qh), (wkh, kh))
                        ):
                            ps0 = qkps.tile([128, 512], f32, tag="qk",
                                            name=f"ps0_{which}")
                            ps1 = qkps.tile([128, 512], f32, tag="qk",
                                            name=f"ps1_{which}")
                            for k in range(6):
                                nc.tensor.matmul(
                                    ps0[:], wt[:, k, :], xTk[k][:, 0:512],
                                    start=(k == 0), stop=(k == 5),
                                )
                                yield
                                nc.tensor.matmul(
                                    ps1[:], wt[:, k, :], xTk[k][:, 512:1024],
                                    start=(k == 0), stop=(k == 5),
                                )
                                yield
                            nc.vector.tensor_copy(dst[:, 0:512], ps0[:])
                            nc.vector.tensor_copy(dst[:, 512:1024], ps1[:])

                    return qh, kh, gen()

                # ---- Phase 1: v = x @ Wv into v_aug ----
                with (
                    tc.tile_pool(name="wv", bufs=1) as wvp,
                    tc.tile_pool(name="vps", bufs=2, space="PSUM") as vps,
                ):
                    wvk = [
                        wvp.tile([128, 1536], f16, name=f"wv{k}")
                        for k in range(6)
                    ]
                    # critical-path DMAs first: the first v matmul needs
                    # xT0 (stationary) + wv0 (moving); head 0's q/k weights
                    # come right after so the qk projection that follows the
                    # v projection is never starved.  bpp goes on the scalar
                    # HWDGE queue - it is only read at the very end.
                    for k in range(6):
                        nc.sync.dma_start(out=xTk[k][:], in_=XT[:, k])
                        nc.sync.dma_start(out=wvk[k][:], in_=WVR[:, k])
                    for hh in range(4):
                        issue_qk_dma(hh)
                    nc.vector.memset(v_aug[:, :, :, 128:129], 1.0)
                    nc.vector.memset(eps_sb[:], EPS)
                    nc.scalar.dma_start(out=bpp_sb[:], in_=BPP[:])
                    qh0, kh0, g0 = emit_qk(0)
                    for _ in g0:
                        pass
                    next_qk = (qh0, kh0)
                    # prefetch the projection weights NOW: these plain DMAs
                    # must not be in flight alongside the XBAR transposes
                    # later (observed data corruption when mixed).
                    for k in range(12):
                        nc.sync.dma_start(out=wpks[k][:], in_=WPR[k])
                    for t in range(8):
                        # one stationary xT chunk serves all three c-ranges
                        pss = [
                            vps.tile([128, 512], f32, tag=f"vps{cr}",
                                     name=f"vps{cr}")
                            for cr in range(3)
                        ]
                        for k in range(6):
                            for cr in range(3):
                                nc.tensor.matmul(
                                    pss[cr][:],
                                    xTk[k][:, t * 128 : (t + 1) * 128],
                                    wvk[k][:, cr * 512 : (cr + 1) * 512],
                                    start=(k == 0),
                                    stop=(k == 5),
                                )
                        for cr in range(3):
                            nc.scalar.copy(
                                v_aug[:, t, 4 * cr : 4 * cr + 4, 0:128],
                                pss[cr][:].rearrange("p (h c) -> p h c", c=128),
                            )

                # ---- Phase 2: attention per head, tail fused per head ----
                with (
                    tc.tile_pool(name="estrip", bufs=3) as ep,
                    tc.tile_pool(name="fin", bufs=4) as fin,
                    tc.tile_pool(name="spool", bufs=2, space="PSUM") as spool,
                    tc.tile_pool(name="avps", bufs=2, space="PSUM") as avps,
                ):

                    av_state = {}

                    def av_chain(ph, pr, e12p, idx):
                        """Emit ONE of the 8 AV accumulation chains of strip
                        (ph, pr).  Chains are emitted interleaved between
                        score pairs so the score LDWs always have AV matmuls
                        behind them to hide their weight-buffer turnaround.
                        idx: 2*c2 + (0: E1 chain, 1: E2 chain + vector tail).
                        """
                        c2, second = divmod(idx, 2)
                        jn = pr * 4 + c2
                        if not second:
                            o = avps.tile([128, 258], f32, tag="o",
                                          name="o_av")
                            av_state[c2] = o
                            # Both accumulation chains share one PSUM bank.
                            # start=True clears has_written bank-wide, so
                            # only the very first matmul may set it.
                            for m in range(8):
                                nc.tensor.matmul(
                                    o[:, 0:129],
                                    e12p[:, m, c2 * 128 : (c2 + 1) * 128],
                                    v_aug[:, m, ph, :],
                                    start=(m == 0),
                                    stop=(m == 7),
                                    skip_group_check=True,
                                )
                            return
                        o = av_state[c2]
                        for m in range(8):
                            nc.tensor.matmul(
                                o[:, 129:258],
                                e12p[:, m, 512 + c2 * 128 : 512 + (c2 + 1) * 128],
                                v_aug[:, m, ph, :],
                                start=False,
                                stop=(m == 7),
                                skip_group_check=True,
                            )
                        # combine + LN stats (VectorE, all per-partition).
                        # Each DVE op may read PSUM at most once, which
                        # forces the two-step r1*u1 - (lam*r2)*u2 form.
                        r1 = fin.tile([128, 1], f32, tag="r1")
                        r2 = fin.tile([128, 1], f32, tag="r2")
                        nc.vector.reciprocal(r1[:], o[:, 128:129])
                        nc.vector.reciprocal(r2[:], o[:, 257:258])
                        t2 = fin.tile([128, 128], f32, tag="t2")
                        nc.vector.tensor_scalar(
                            t2[:], o[:, 129:257], r2[:], float(lam),
                            op0=OP.mult, op1=OP.mult,
                        )
                        nc.vector.scalar_tensor_tensor(
                            o_ln[:, ph, jn, :],
                            o[:, 0:128],
                            r1[:],
                            t2[:],
                            op0=OP.mult,
                            op1=OP.subtract,
                        )
                        st6 = fin.tile([128, 6], f32, tag="st6")
                        nc.vector.bn_stats(st6[:], o_ln[:, ph, jn, :])
                        nc.vector.bn_aggr(stats_sb[:, ph, jn, :], st6[:])

                    def head_tail(ph):
                        """rsqrt, LN apply, transpose for a finished head.
                        rs = exp(-0.5 * ln(var + eps)); Exp and Ln share one
                        pinned table set, so no reload happens here."""
                        nc.scalar.activation(
                            sdbuf[:, ph, :], stats_sb[:, ph, :, 1],
                            AF.Ln, bias=eps_sb[:],
                        )
                        nc.scalar.activation(
                            rsbuf[:, ph, :], sdbuf[:, ph, :], AF.Exp,
                            scale=-0.5,
                        )
                        for jn in range(8):
                            nc.vector.tensor_scalar(
                                o_ln[:, ph, jn, :],
                                o_ln[:, ph, jn, :],
                                stats_sb[:, ph, jn, 0:1],
                                rsbuf[:, ph, jn : jn + 1],
                                op0=OP.subtract,
                                op1=OP.mult,
                            )
                        # transpose the whole finished head [tok, (jn, c)] ->
                        # [c, (jn, tok)] in ONE XBAR DMA (blocked transpose:
                        # out[c, jn, t] = in[t, jn, c]) - TensorE never sees
                        # it, and one instruction per head keeps the sync
                        # queue issue cost negligible.
                        nc.sync.dma_start(
                            out=o_lnT[:, ph, :].rearrange(
                                "p (a b) -> p a b", b=128
                            ),
                            in_=o_ln[:, ph],
                            transpose=True,
                        )

                    pending = None  # (head, strip, e12) awaiting its AV
                    for h in range(12):
                        qh, kh = next_qk
                        qk_gen = None
                        if h + 1 < 12:
                            if h + 4 < 12:
                                issue_qk_dma(h + 4)
                            # next head's q/k projection: its 24 matmuls are
                            # pulled 1-2 per m-slot across this head's 16
                            # slots, keeping the PE slot time >= the exp time
                            # so the score-PSUM reuse never stalls on ScalarE
                            nqh, nkh, qk_gen = emit_qk(h + 1)
                            next_qk = (nqh, nkh)

                        slot = 0
                        for r in range(2):
                            e12 = ep.tile([128, 8, 1024], f16, tag="e")
                            nsl = slice(r * 512, (r + 1) * 512)
                            for m in range(8):
                                msl = slice(m * 128, (m + 1) * 128)
                                # The two score matmuls must hit different
                                # PSUM banks (concurrent row-group writes to
                                # one bank fault); one exp covers both.
                                sp = spool.tile([128, 2, 512], f32, tag="s")
                                nc.tensor.matmul(
                                    sp[:, 0, :], kh[0:64, msl], qh[0:64, nsl],
                                    start=True, stop=True,
                                )
                                nc.tensor.matmul(
                                    sp[:, 1, :], kh[64:128, msl],
                                    qh[64:128, nsl],
                                    start=True, stop=True,
                                )
                                if pending is not None:
                                    av_chain(pending[0], pending[1],
                                             pending[2], m)
                                if qk_gen is not None:
                                    for _ in range(1 + (slot % 2)):
                                        next(qk_gen, None)
                                slot += 1
                                nc.scalar.activation(
                                    e12[:, m, :].rearrange("p (a b) -> p a b", a=2),
                                    sp[:],
                                    AF.Exp,
                                    scale=SCALE,
                                )
                            if dbg and h == 0 and r == 0:
                                nc.sync.dma_start(out=DE12[:], in_=e12[:])
                            if pending is not None and pending[1] == 1:
                                head_tail(pending[0])
                            pending = (h, r, e12)
                        if qk_gen is not None:
                            for _ in qk_gen:
                                pass
                        if dbg and h == 0:
                            nc.sync.dma_start(out=DQH[:], in_=qh[:])
                            nc.sync.dma_start(out=DKH[:], in_=kh[:])

                    # drain: AV + tail of the very last strip
                    for m in range(8):
                        av_chain(pending[0], pending[1], pending[2], m)
                    head_tail(pending[0])
                    if dbg:
                        nc.sync.dma_start(out=DVAUG[:], in_=v_aug[:])
                        nc.sync.dma_start(out=DOLN[:], in_=o_ln[:])
                        nc.sync.dma_start(out=DSTATS[:], in_=stats_sb[:])

                _pools.close()

            # longA (xT, v_aug) released here.
            if dbg:
                nc.sync.dma_start(out=DOLNT[:], in_=o_lnT[:])
            # ---- Phase 3: final projection, one [128, 512] tile at a time.
            # Wave A (first 6 tiles) streams heads 0..10 and DEFERS head 11,
            # so the PE has ~14us of proj work queued before it ever needs
            # the last head's transpose; wave B then runs all heads k-inner
            # per tile so evictions/output DMAs pipeline behind the stream.
            with tc.tile_pool(name="tail", bufs=1) as tailp:
                fout = tailp.tile([128, 6, 1024], f32)
                with (
                    tc.tile_pool(name="fpsA", bufs=1, space="PSUM") as fpsA,
                    tc.tile_pool(name="fpsB", bufs=2, space="PSUM") as fpsB,
                ):
                    tiles = [(mc, nr2) for mc in range(6) for nr2 in range(2)]

                    def evict(fs, mc, nr2, scalar_side=False):
                        # Alternate evict+DMA between the DVE/sync and
                        # ScalarE/scalar queue pairs so the final drain
                        # pipelines across two queues.  AF.Copy + bias AP
                        # makes ScalarE do the bias add.
                        nsl2 = slice(nr2 * 512, (nr2 + 1) * 512)
                        if scalar_side:
                            nc.scalar.activation(
                                fout[:, mc, nsl2], fs[:], AF.Identity,
                                bias=bpp_sb[:, mc : mc + 1],
                            )
                            nc.scalar.dma_start(
                                out=OUT[:, mc, nsl2],
                                in_=fout[:, mc, nsl2],
                            )
                        else:
                            nc.vector.tensor_scalar(
                                fout[:, mc, nsl2],
                                fs[:],
                                bpp_sb[:, mc : mc + 1],
                                None,
                                op0=OP.add,
                            )
                            nc.sync.dma_start(
                                out=OUT[:, mc, nsl2],
                                in_=fout[:, mc, nsl2],
                            )

                    fsA = [
                        fpsA.tile([128, 512], f32, tag=f"fa{t}", name=f"fA{t}")
                        for t in range(6)
                    ]
                    for t in range(6):
                        mc, nr2 = tiles[t]
                        nsl2 = slice(nr2 * 512, (nr2 + 1) * 512)
                        ks = [(k0 + t) % 11 for k0 in range(11)]
                        for i, k in enumerate(ks):
                            nc.tensor.matmul(
                                fsA[t][:],
                                wpks[k][:, mc * 128 : (mc + 1) * 128],
                                o_lnT[:, k, nsl2],
                                start=(i == 0),
                                stop=False,
                            )
                    for t in range(6):
                        mc, nr2 = tiles[t]
                        nsl2 = slice(nr2 * 512, (nr2 + 1) * 512)
                        nc.tensor.matmul(
                            fsA[t][:],
                            wpks[11][:, mc * 128 : (mc + 1) * 128],
                            o_lnT[:, 11, nsl2],
                            start=False,
                            stop=True,
                        )
                        evict(fsA[t], mc, nr2)
                    for t in range(6, 12):
                        mc, nr2 = tiles[t]
                        nsl2 = slice(nr2 * 512, (nr2 + 1) * 512)
                        fs = fpsB.tile([128, 512], f32, tag="fb",
                                       name=f"fB{t}")
                        ks = [(k0 + t) % 12 for k0 in range(12)]
                        for i, k in enumerate(ks):
                            nc.tensor.matmul(
                                fs[:],
                                wpks[k][:, mc * 128 : (mc + 1) * 128],
                                o_lnT[:, k, nsl2],
                                start=(i == 0),
                                stop=(i == 11),
                            )
                        evict(fs, mc, nr2, scalar_side=(t % 2 == 1))

    nc.compile()
    return nc


def _host_prep(x, Wq, Wk, Wv, gamma, beta, Wp, bp):
    x = np.ascontiguousarray(np.asarray(x, np.float32))
    Wq = np.asarray(Wq, np.float32)
    Wk = np.asarray(Wk, np.float32)
    Wv = np.asarray(Wv, np.float32)
    Wp = np.asarray(Wp, np.float32)
    bp = np.asarray(bp, np.float32)
    gamma = np.asarray(gamma, np.float32)
    beta = np.asarray(beta, np.float32)

    # xT per batch: [128, 6, 1024] with [p, k, n] = x[b, n, k*128+p]
    xTr = np.ascontiguousarray(
        x.transpose(0, 2, 1).reshape(B, 6, 128, N).transpose(0, 2, 1, 3)
    ).astype(np.float16)

    # W[qk]R: [12, 128, 6, 128] with [h, p, k, c] = W[k*128+p, h*128+c]
    def wqk_r(W):
        return np.ascontiguousarray(
            W.reshape(6, 128, 12, 128).transpose(2, 1, 0, 3)
        )

    WqR = wqk_r(Wq).astype(np.float16)
    WkR = wqk_r(Wk).astype(np.float16)
    # WvR: [128, 6, 1536] with [p, k, c] = Wv[k*128+p, c]
    WvR = np.ascontiguousarray(
        Wv.reshape(6, 128, 2 * C).transpose(1, 0, 2)
    ).astype(np.float16)
    # Fold gamma and the (1 - lambda_init) scale into Wp; beta into the bias.
    gfull = np.tile(gamma, H)  # [1536]
    Wpg = Wp * (OUT_SCALE * gfull)[:, None]
    bpp = bp + OUT_SCALE * (np.tile(beta, H) @ Wp)
    WpR = np.ascontiguousarray(Wpg.reshape(12, 128, C)).astype(np.float16)
    bppR = np.ascontiguousarray(bpp.reshape(6, 128).T)  # [128, 6]
    return xTr, WqR, WkR, WvR, WpR, bppR


def kernel(x, Wq, Wk, Wv, lam, gamma, beta, Wp, bp):
    global LAST_EXEC_NS
    import os

    from concourse.bass_utils import run_bass_kernel_spmd

    lam_f = float(np.asarray(lam))
    xTr, WqR, WkR, WvR, WpR, bppR = _host_prep(
        x, Wq, Wk, Wv, gamma, beta, Wp, bp
    )

    key = lam_f
    if key not in _BUILD_CACHE:
        _BUILD_CACHE[key] = _build(lam_f)
    nc = _BUILD_CACHE[key]

    in_maps = [
        {
            "xT": xTr[b],
            "WqR": WqR,
            "WkR": WkR,
            "WvR": WvR,
            "WpR": WpR,
            "bpp": bppR,
        }
        for b in range(B)
    ]

    trace = bool(os.environ.get("BASS_KERNEL_TRACE"))
    if trace:
        from concourse import bass_utils as _bu

        _bu.upload_artifacts = lambda tmpdir: "local://" + tmpdir
    res = run_bass_kernel_spmd(
        nc, in_maps, list(range(B)), trace=trace,
        **({"trace_cores": list(range(B))} if trace else {}),
    )
    LAST_EXEC_NS = res.exec_time_ns

    out = np.empty((B, N, C), np.float32)
    for b in range(B):
        outT = res.results[b]["outT"]  # [128, 6, 1024]
        out[b] = outT.transpose(2, 1, 0).reshape(N, C)
    return out
